# revision 9
# baseline (speedup 1.0000x reference)
"""Trainium2 Bass kernel for nn_ConsecutiveIntentUnit (set-encoder + GRU + combine).

Sharding: data-parallel over the batch dim across 8 NeuronCores (128 rows each),
weights replicated. Per core, everything runs in a "transposed" layout with the
embedding dim E=128 on SBUF partitions and the local batch (128) on the free dim:

  - x is fed pre-transposed as xT [E, S*B_local] (s-major columns), bf16.
  - GRU gates are computed in PSUM: per step one 2KB bank holds
    [a_r | a_z | gi_n | gh_n] (4 x 128 fp32 columns). The three gi matmuls
    (W_ihT stationary, xT_s moving) write with start=True; the r/z recurrent
    matmuls (W_hhT stationary, h_T moving) accumulate start=False into the same
    regions, so gi_g + gh_g needs no extra adds. gh_n stays separate because the
    PyTorch GRU applies r before adding it.
  - Biases ride along for free: sigmoid/tanh get per-partition bias APs on the
    scalar engine, and the n-gate biases fold into two fused
    scalar_tensor_tensor ops on the vector engine.
  - h' = h + (1-z)*(n-h), with (1-z) computed directly as sigmoid(-a_z - b_z).
  - mean-pool over S is a fp32 add-tree over the already-resident xT (first
    level on GPSIMD, rest on VectorE); the 1/S scale is folded into W1 on host.
  - set encoder + combine are a handful of matmuls at the end; b2 is folded
    into an effective combine bias (b_c + W_c[:, :E] @ b2) on host.

The kernel returns the full [1024, 128] fp32 output; per-core [E, B] results
are transposed and concatenated on host.
"""

import time

import numpy as np
import ml_dtypes
from contextlib import ExitStack

import concourse.bacc as bacc
import concourse.mybir as mybir
import concourse.tile as tile
from concourse.bass_utils import run_bass_kernel_spmd
from concourse.mybir import AluOpType as alu, ActivationFunctionType as actf

B, S, E, H = 1024, 200, 128, 256
NCORES = 8
BL = B // NCORES  # 128 batch rows per core

bf16 = mybir.dt.bfloat16
f32 = mybir.dt.float32
BF = ml_dtypes.bfloat16


def build(s_steps=S, bl=BL):
    nc = bacc.Bacc("TRN2", target_bir_lowering=False, debug=False)

    xt_d = nc.dram_tensor("xt", [E, s_steps * bl], bf16, kind="ExternalInput").ap()
    wih_d = nc.dram_tensor("wih", [E, 3 * E], bf16, kind="ExternalInput").ap()
    whh_d = nc.dram_tensor("whh", [E, 3 * E], bf16, kind="ExternalInput").ap()
    wenc_d = nc.dram_tensor("wenc", [128, 768], bf16, kind="ExternalInput").ap()
    bias_d = nc.dram_tensor("biases", [128, 8], f32, kind="ExternalInput").ap()
    out_d = nc.dram_tensor("out_t", [E, bl], f32, kind="ExternalOutput").ap()

    with tile.TileContext(nc) as tc, ExitStack() as ctx:
        cpool = ctx.enter_context(tc.tile_pool(name="consts", bufs=1))
        xpool = ctx.enter_context(tc.tile_pool(name="x", bufs=1))
        hpool = ctx.enter_context(tc.tile_pool(name="h", bufs=2))
        gpool = ctx.enter_context(tc.tile_pool(name="g", bufs=2))
        ppool = ctx.enter_context(tc.tile_pool(name="ps", bufs=6, space="PSUM"))

        wih = cpool.tile([E, 3 * E], bf16)
        nc.sync.dma_start(wih[:], wih_d[:])
        whh = cpool.tile([E, 3 * E], bf16)
        nc.sync.dma_start(whh[:], whh_d[:])
        wenc = cpool.tile([128, 768], bf16)
        nc.sync.dma_start(wenc[:], wenc_d[:])
        bias = cpool.tile([128, 7], f32)
        nc.sync.dma_start(bias[:], bias_d[:])
        b_r = bias[:, 0:1]
        nb_z = bias[:, 1:2]
        b_hhn = bias[:, 2:3]
        b_ihn = bias[:, 3:4]
        b1a = bias[:, 4:5]
        b1b = bias[:, 5:6]
        bce = bias[:, 6:7]

        xt = xpool.tile([E, s_steps * bl], bf16)
        nch = min(8, s_steps)
        bounds = [s_steps * c // nch * bl for c in range(nch + 1)]
        for c in range(nch):
            nc.sync.dma_start(xt[:, bounds[c]:bounds[c + 1]],
                              xt_d[:, bounds[c]:bounds[c + 1]])

        mm = nc.tensor.matmul
        h_prev = None
        for t in range(s_steps):
            first = t == 0
            bank = ppool.tile([128, 512], f32, tag="bank")
            xs = xt[:, t * bl:(t + 1) * bl]
            # One accumulation group per PSUM bank: start=True zeroes the whole
            # 2KB bank, later matmuls overwrite-or-accumulate per element via
            # has_written, the last one closes the group.
            mm(bank[:, 0:128], wih[:, 0:128], xs, start=True, stop=False)
            mm(bank[:, 128:256], wih[:, 128:256], xs, start=False, stop=False)
            mm(bank[:, 256:384], wih[:, 256:384], xs, start=False, stop=first)
            if not first:
                mm(bank[:, 384:512], whh[:, 256:384], h_prev[:], start=False, stop=False)
                mm(bank[:, 0:128], whh[:, 0:128], h_prev[:], start=False, stop=False)
                mm(bank[:, 128:256], whh[:, 128:256], h_prev[:], start=False, stop=True)

            r = gpool.tile([E, bl], bf16, tag="r")
            nc.scalar.activation(r[:], bank[:, 0:128], actf.Sigmoid, bias=b_r)
            zp = gpool.tile([E, bl], bf16, tag="zp")  # zp = 1 - z
            nc.scalar.activation(zp[:], bank[:, 128:256], actf.Sigmoid,
                                 bias=nb_z, scale=-1.0)
            tmp = gpool.tile([E, bl], bf16, tag="tmp")  # r * (gh_n + b_hhn)
            if first:
                nc.vector.tensor_scalar(tmp[:], r[:], b_hhn, None, alu.mult)
            else:
                nc.vector.scalar_tensor_tensor(tmp[:], bank[:, 384:512], b_hhn,
                                               r[:], alu.add, alu.mult)
            npre = gpool.tile([E, bl], bf16, tag="npre")  # gi_n + b_ihn + tmp
            nc.vector.scalar_tensor_tensor(npre[:], bank[:, 256:384], b_ihn,
                                           tmp[:], alu.add, alu.add)
            n = gpool.tile([E, bl], bf16, tag="n")
            nc.scalar.activation(n[:], npre[:], actf.Tanh)
            h_new = hpool.tile([E, bl], bf16, tag="h")
            if first:
                nc.vector.tensor_mul(h_new[:], zp[:], n[:])
            else:
                u = gpool.tile([E, bl], bf16, tag="u")  # n - h
                nc.vector.scalar_tensor_tensor(u[:], h_prev[:], -1.0, n[:],
                                               alu.mult, alu.add)
                v = gpool.tile([E, bl], bf16, tag="v")  # (1-z)*(n-h)
                nc.vector.tensor_mul(v[:], zp[:], u[:])
                nc.vector.tensor_add(h_new[:], h_prev[:], v[:])
            h_prev = h_new

        # ---- mean-pool over s: fp32 add tree over xT (s-major columns) ----
        extras = []  # [E, bl] fp32 leftovers
        w = s_steps
        if w == 1:
            pooled = gpool.tile([E, bl], f32, tag="pool_f")
            nc.vector.tensor_copy(pooled[:], xt[:, 0:bl])
            cur = pooled
        else:
            w2 = w // 2
            l1 = xpool.tile([E, (w2 + 1) * bl], f32, tag="l1")
            nc.gpsimd.tensor_add(l1[:, 0:w2 * bl], xt[:, 0:w2 * bl],
                                 xt[:, w2 * bl:2 * w2 * bl])
            if w % 2:
                ex = gpool.tile([E, bl], f32, tag="pool_ex0")
                nc.vector.tensor_copy(ex[:], xt[:, 2 * w2 * bl:(2 * w2 + 1) * bl])
                extras.append(ex)
            cur, w = l1, w2
            scr = xpool.tile([E, ((w // 2) + 1) * bl], f32, tag="l2")
            flip = False
            while w > 1:
                w2 = w // 2
                dst = scr if not flip else l1
                nc.vector.tensor_add(dst[:, 0:w2 * bl], cur[:, 0:w2 * bl],
                                     cur[:, w2 * bl:2 * w2 * bl])
                if w % 2:
                    ex = gpool.tile([E, bl], f32, tag=f"pool_ex{len(extras)}")
                    nc.vector.tensor_copy(ex[:], cur[:, 2 * w2 * bl:(2 * w2 + 1) * bl])
                    extras.append(ex)
                cur, w = dst, w2
                flip = not flip
        for i, ex in enumerate(extras):
            acc = gpool.tile([E, bl], f32, tag=f"pool_acc{i}")
            nc.vector.tensor_add(acc[:], cur[:, 0:bl], ex[:])
            cur = acc
        pooled_bf = gpool.tile([E, bl], bf16, tag="pool_bf")
        nc.vector.tensor_copy(pooled_bf[:], cur[:, 0:bl])

        # ---- set encoder + combine ----
        pe1 = ppool.tile([128, 512], f32, tag="bank")
        mm(pe1[:, 0:128], wenc[:, 0:128], pooled_bf[:], start=True, stop=False)
        mm(pe1[:, 128:256], wenc[:, 128:256], pooled_bf[:], start=False, stop=True)
        s1a = gpool.tile([128, bl], bf16, tag="s1a")
        nc.scalar.activation(s1a[:], pe1[:, 0:128], actf.Relu, bias=b1a)
        s1b = gpool.tile([128, bl], bf16, tag="s1b")
        nc.scalar.activation(s1b[:], pe1[:, 128:256], actf.Relu, bias=b1b)
        pe2 = ppool.tile([128, 512], f32, tag="bank")
        mm(pe2[:, 0:128], wenc[:, 256:384], s1a[:], start=True, stop=False)
        mm(pe2[:, 0:128], wenc[:, 384:512], s1b[:], start=False, stop=True)
        s2 = gpool.tile([128, bl], bf16, tag="s2")
        nc.scalar.activation(s2[:], pe2[:, 0:128], actf.Copy)
        pe3 = ppool.tile([128, 512], f32, tag="bank")
        mm(pe3[:, 0:128], wenc[:, 512:640], s2[:], start=True, stop=False)
        mm(pe3[:, 0:128], wenc[:, 640:768], h_prev[:], start=False, stop=True)
        outt = gpool.tile([E, bl], f32, tag="outt")
        nc.scalar.activation(outt[:], pe3[:, 0:128], actf.Tanh, bias=bce)
        nc.sync.dma_start(out_d[:], outt[:])

    nc.compile()
    return nc


def prep_weights(W1, b1, W2, b2, W_ih, b_ih, W_hh, b_hh, W_c, b_c, s_steps=S):
    f = np.float32
    wih = np.ascontiguousarray(np.asarray(W_ih, f).T).astype(BF)      # [E, 3E]
    whh = np.ascontiguousarray(np.asarray(W_hh, f).T).astype(BF)      # [E, 3E]
    w1t = (np.asarray(W1, f) / float(s_steps)).T                       # [E, H]
    w2t = np.asarray(W2, f).T                                          # [H, E]
    wct = np.asarray(W_c, f).T                                         # [2E, E]
    wenc = np.concatenate(
        [w1t[:, 0:128], w1t[:, 128:256],
         w2t[0:128, :], w2t[128:256, :],
         wct[0:128, :], wct[128:256, :]], axis=1).astype(BF)           # [128, 768]
    bce = np.asarray(b_c, f) + np.asarray(W_c, f)[:, :E] @ np.asarray(b2, f)
    biases = np.stack(
        [np.asarray(b_ih, f)[:E] + np.asarray(b_hh, f)[:E],
         -(np.asarray(b_ih, f)[E:2 * E] + np.asarray(b_hh, f)[E:2 * E]),
         np.asarray(b_hh, f)[2 * E:],
         np.asarray(b_ih, f)[2 * E:],
         np.asarray(b1, f)[0:128], np.asarray(b1, f)[128:256],
         bce], axis=1).astype(f)                                       # [128, 7]
    return wih, whh, wenc, biases


def prep_x_core(x_core, s_steps=S, bl=BL):
    # x_core [bl, S, E] fp32 -> xT [E, S*bl] bf16 with s-major columns
    xt = np.ascontiguousarray(
        np.asarray(x_core, np.float32).transpose(2, 1, 0)).astype(BF)
    return xt.reshape(E, s_steps * bl)


_NC_CACHE = {}


def _prep_in_maps(inputs):
    x = np.asarray(inputs["item_embeddings"], np.float32)
    wih, whh, wenc, biases = prep_weights(
        inputs["W1"], inputs["b1"], inputs["W2"], inputs["b2"],
        inputs["W_ih"], inputs["b_ih"], inputs["W_hh"], inputs["b_hh"],
        inputs["W_c"], inputs["b_c"])
    in_maps = []
    for c in range(NCORES):
        in_maps.append({
            "xt": prep_x_core(x[c * BL:(c + 1) * BL]),
            "wih": wih, "whh": whh, "wenc": wenc, "biases": biases,
        })
    return in_maps


def time_kernel(n_lo=1, n_hi=9, trials=5, **inputs):
    """Estimate on-device NEFF execution time by timing jitted chains of
    n_lo vs n_hi back-to-back executions and taking the slope, which cancels
    the constant PJRT/tunnel dispatch overhead."""
    import jax
    from jax.sharding import Mesh, PartitionSpec
    from jax.experimental.shard_map import shard_map
    from concourse import bass2jax
    import concourse.mybir as mybir_

    if "nc" not in _NC_CACHE:
        _NC_CACHE["nc"] = build()
    nc = _NC_CACHE["nc"]
    bass2jax.install_neuronx_cc_hook()
    in_maps = _prep_in_maps(inputs)

    part_name = nc.partition_id_tensor.name if nc.partition_id_tensor else None
    in_names, out_names, out_avals = [], [], []
    for alloc in nc.m.functions[0].allocations:
        if not isinstance(alloc, mybir.MemoryLocationSet):
            continue
        name = alloc.memorylocations[0].name
        if alloc.kind == "ExternalInput":
            if name != part_name:
                in_names.append(name)
        elif alloc.kind == "ExternalOutput":
            out_names.append(name)
            out_avals.append(jax.core.ShapedArray(
                tuple(alloc.tensor_shape), mybir.dt.np(alloc.dtype)))
    n_params = len(in_names)
    all_names = in_names + out_names + ([part_name] if part_name else [])

    def _body(*args):
        operands = list(args)
        if part_name:
            operands.append(bass2jax.partition_id_tensor())
        outs = bass2jax._bass_exec_p.bind(
            *operands,
            out_avals=tuple(out_avals),
            in_names=tuple(all_names),
            out_names=tuple(out_names),
            lowering_input_output_aliases=(),
            sim_require_finite=True, sim_require_nnan=True, nc=nc)
        return tuple(outs)

    devices = jax.devices()[:NCORES]
    mesh = Mesh(np.asarray(devices), ("core",))
    nargs = n_params + len(out_names)
    fn = jax.jit(shard_map(
        _body, mesh=mesh,
        in_specs=(PartitionSpec("core"),) * nargs,
        out_specs=(PartitionSpec("core"),) * len(out_names),
        check_rep=False), keep_unused=True)

    concat_in = [
        jax.device_put(np.concatenate(
            [np.asarray(in_maps[c][nm]) for c in range(NCORES)], axis=0))
        for nm in in_names]
    zeros = [jax.device_put(np.zeros((NCORES * av.shape[0], *av.shape[1:]),
                                     av.dtype)) for av in out_avals]

    jax.block_until_ready(fn(*concat_in, *zeros))  # warmup/compile

    # Async-queue slope: launch n calls back-to-back before blocking; the
    # per-call wall-clock difference vs 1 call cancels dispatch overhead.
    def run_n(n):
        best = float("inf")
        for _ in range(trials):
            t0 = time.perf_counter()
            outs = None
            for _ in range(n):
                outs = fn(*concat_in, *zeros)
            jax.block_until_ready(outs)
            best = min(best, time.perf_counter() - t0)
        return best
    t_lo, t_hi = run_n(n_lo), run_n(n_hi)
    est = max((t_hi - t_lo) / (n_hi - n_lo), 0.0)
    print(f"  timing: T({n_lo})={t_lo*1e3:.3f}ms T({n_hi})={t_hi*1e3:.3f}ms "
          f"slope={est*1e6:.0f}us/call")
    return int(est * 1e9)


def kernel(**inputs):
    if "nc" not in _NC_CACHE:
        _NC_CACHE["nc"] = build()
    nc = _NC_CACHE["nc"]

    x = np.asarray(inputs["item_embeddings"], np.float32)
    wih, whh, wenc, biases = prep_weights(
        inputs["W1"], inputs["b1"], inputs["W2"], inputs["b2"],
        inputs["W_ih"], inputs["b_ih"], inputs["W_hh"], inputs["b_hh"],
        inputs["W_c"], inputs["b_c"])

    in_maps = []
    for c in range(NCORES):
        in_maps.append({
            "xt": prep_x_core(x[c * BL:(c + 1) * BL]),
            "wih": wih, "whh": whh, "wenc": wenc, "biases": biases,
        })
    res = run_bass_kernel_spmd(nc, in_maps, core_ids=list(range(NCORES)))

    out = np.empty((B, E), np.float32)
    for c in range(NCORES):
        out[c * BL:(c + 1) * BL, :] = res.results[c]["out_t"].T
    return out
